# revision 1
# baseline (speedup 1.0000x reference)
"""Trainium2 Bass kernel for nn_MetaUpSample (2x meta-upsample, 3x3 dynamic filters).

out[b,ho,wo,f] = sum_k patches[b,ho,wo,k] * meta_w[b,ho,wo,k*3+f]
  patches[b,ho,wo,(dk0,dk1,c)] = x_pad[b, ho//2+dk0, wo//2+dk1, c]

Sharding: 8 cores, core ci handles b = ci//2, ho in [(ci%2)*64, (ci%2)*64+64).
meta_w (432 MiB total) is the dominant HBM stream (~56.6 MiB/core); the kernel
streams it once (partition = wo) and fuses multiply+reduce in single DVE
scalar_tensor_tensor ops: out = (mw * 1.0) * patch, accum_out = per-partition
sum over K. 3 ops per output row (one per filter), 192 per core.

Host side pre-builds (a) the duplicated patch-row tensor xrb (x is only 4 MiB:
xrb[wo, hp, :] = the 192 floats of padded x row hp that pixel column wo needs),
(b) an F-major relayout of meta_w so the strided operand becomes contiguous,
and un-transposes the [wo, (ho,f)] device output. The device graph is just:
  1 xrb DMA (ACT ring) + 16 x (3.54MiB meta_w DMA + 12 fused ops) + 1 out DMA.
"""
from contextlib import ExitStack

import numpy as np

import concourse.bass as bass
import concourse.mybir as mybir
from concourse.bass_utils import run_bass_kernel_spmd

B, H, W, C = 4, 64, 64, 64
HO, WO, F = 128, 128, 3
KS = 3
K = KS * KS * C            # 576
QF = K * F                 # 1728 meta_w channels
RW = KS * C                # 192 floats per patch row (dk1, c)
N_CORES = 8
CORES_PER_B = N_CORES // B         # 2
HO_PC = HO // CORES_PER_B          # 64 output rows per core
NHS = HO_PC // 2                   # 32 hs tiles per core
NROWS = NHS + 2                    # 34 cached padded x rows per core

import os

# Tuned on HW (TRN2, 8 cores). Notes from the sweep:
#  - deeper prefetch (NBUF*RPT >= ~14 rows) consistently REGRESSES (SBUF
#    port/bank contention between the DMA writes and DVE reads);
#  - F-major host layout makes the STT in0 read contiguous: 834 -> 685 ns/op;
#  - mixed small/large tile ramp schedules regress (DMA is the bottleneck
#    engine; small tiles starve it).
NBUF = int(os.environ.get("K_NBUF", "3"))    # meta_w double-buffer slots
RPT = int(os.environ.get("K_RPT", "4"))      # meta_w rows per DMA tile
NSCR = int(os.environ.get("K_NSCR", "2"))    # DVE scratch ring slots
SELF_WAITS = os.environ.get("K_WAITS", "0") == "1"
FMAJOR = os.environ.get("K_FMAJOR", "1") == "1"  # host-transpose mw to [f,k]
XH = RPT + 1  # xrb head rows loaded in the first chunk (covers tile 0)

f32 = mybir.dt.float32

if os.environ.get("K_RAMP", "0") == "1":
    SCHED = [1, 1, 2] + [RPT] * ((HO_PC - 8) // RPT) + [2, 1, 1]
elif os.environ.get("K_TAIL", "1") == "1":
    # small TAIL tiles only: after the last meta_w byte lands, DVE has just a
    # 1-row tile (3 ops, ~2us) left instead of a 4-row one (~8us). Small HEAD
    # tiles regress (they starve the DMA stream early), so the head stays 4-row.
    SCHED = [RPT] * ((HO_PC - 4) // RPT) + [2, 1, 1]
else:
    SCHED = [RPT] * (HO_PC // RPT)
assert sum(SCHED) == HO_PC
NT = len(SCHED)
ROW0 = [sum(SCHED[:t]) for t in range(NT)]          # first ho row of tile t
OPS0 = [ROW0[t] * F for t in range(NT)]             # DVE ops before tile t
MAXR = max(SCHED)

_CACHED = None


def _build_nc():
    # Cross-engine ordering is fully explicit via semaphores below; the
    # remaining same-engine WAW (DVE scratch ring) is safe on HW because DVE
    # drains its pipe between ops, so skip the detector's extra waits.
    nc = bass.Bass(detect_race_conditions=False)
    mw_d = nc.declare_dram_parameter("mw", [HO_PC, WO, QF], f32, isOutput=False)
    xrb_d = nc.declare_dram_parameter("xrb", [WO, NROWS * RW], f32, isOutput=False)
    out_d = nc.declare_dram_parameter("out", [WO, HO_PC * F], f32, isOutput=True)

    with ExitStack() as ctx:
        xrow = ctx.enter_context(nc.sbuf_tensor([WO, NROWS * RW], f32))
        mwbuf = ctx.enter_context(nc.sbuf_tensor([WO, NBUF * MAXR * QF], f32))
        scr_v = ctx.enter_context(nc.sbuf_tensor([WO, NSCR * K], f32))
        out_sb = ctx.enter_context(nc.sbuf_tensor([WO, HO_PC * F], f32))
        slot_sem = [ctx.enter_context(nc.semaphore(f"slot{j}")) for j in range(NBUF)]
        misc_sem = ctx.enter_context(nc.semaphore("misc"))
        cmp_v = ctx.enter_context(nc.semaphore("cmp_v"))   # DVE fused ops done
        block = ctx.enter_context(nc.Block())

        def slot_ap(j, rows):
            base = j * MAXR * QF
            return mwbuf[:, base : base + rows * QF]

        @block.sync
        def _(sync):
            for i in range(NT):
                j = i % NBUF
                rows, row0 = SCHED[i], ROW0[i]
                if i >= NBUF:
                    # DVE finished reading the slot's previous tile
                    prev = i - NBUF
                    sync.wait_ge(cmp_v, OPS0[prev] + SCHED[prev] * F)
                sync.dma_start(
                    out=slot_ap(j, rows).rearrange("p (h q) -> p h q", h=rows),
                    in_=mw_d[row0 : row0 + rows].rearrange("h w q -> w h q"),
                ).then_inc(slot_sem[j], 16)
            # overlap the bulk of the output store with the tail tiles;
            # only a 12KB final piece remains after the last compute op
            sync.wait_ge(cmp_v, 60 * F)
            sync.dma_start(
                out=out_d[:, : 60 * F], in_=out_sb[:, : 60 * F]
            ).then_inc(misc_sem, 16)
            sync.wait_ge(cmp_v, HO_PC * F)
            sync.dma_start(
                out=out_d[:, 60 * F :], in_=out_sb[:, 60 * F :]
            ).then_inc(misc_sem, 16)

        @block.scalar
        def _(scalar):
            # xrb on the ACT HWDGE ring so it doesn't head-of-line block the
            # meta_w stream on the SP ring; head chunk first so DVE can start
            # tile 0 after ~480KB instead of 3.3MB.
            scalar.dma_start(
                out=xrow[:, : XH * RW], in_=xrb_d[:, : XH * RW]
            ).then_inc(misc_sem, 16)
            scalar.dma_start(
                out=xrow[:, XH * RW :], in_=xrb_d[:, XH * RW :]
            ).then_inc(misc_sem, 16)

        @block.vector
        def _(vector):
            vector.wait_ge(misc_sem, 16)
            nv = 0
            xrow_full_waited = False
            for i in range(NT):
                j, p = i % NBUF, i // NBUF
                rows = SCHED[i]
                if not xrow_full_waited and (ROW0[i] + rows - 1) // 2 + 2 >= XH:
                    vector.wait_ge(misc_sem, 32)  # rest of xrow loaded
                    xrow_full_waited = True
                vector.wait_ge(slot_sem[j], 16 * (p + 1))
                if FMAJOR:
                    mw4 = slot_ap(j, rows).rearrange(
                        "p (h f k) -> p h f k", h=rows, f=F
                    )
                else:
                    mw4 = slot_ap(j, rows).rearrange(
                        "p (h k f) -> p h k f", h=rows, f=F
                    )
                for r in range(rows):
                    ho = ROW0[i] + r
                    win = xrow[:, (ho // 2) * RW : (ho // 2) * RW + KS * RW]
                    for f in range(F):
                        if SELF_WAITS and nv >= NSCR:
                            vector.wait_ge(cmp_v, nv - NSCR + 1)
                        vector.scalar_tensor_tensor(
                            out=scr_v[:, (nv % NSCR) * K : (nv % NSCR + 1) * K],
                            in0=mw4[:, r, f, :] if FMAJOR else mw4[:, r, :, f],
                            scalar=1.0,
                            in1=win,
                            op0=mybir.AluOpType.mult,
                            op1=mybir.AluOpType.mult,
                            accum_out=out_sb[:, ho * F + f : ho * F + f + 1],
                        ).then_inc(cmp_v, 1)
                        nv += 1

    return nc


def _prep_xrb(x):
    """Per-core duplicated patch-row tensors.

    xrb[ci][wo, hpl*RW + dk1*C + c] = x_pad[b, hs0+hpl, wo//2 + dk1, c]
    where x_pad has 1 zero row/col of padding on each side.
    """
    from numpy.lib.stride_tricks import sliding_window_view

    out = []
    for ci in range(N_CORES):
        b, hs0 = ci // CORES_PER_B, (ci % CORES_PER_B) * NHS
        xp = np.pad(x[b], ((1, 1), (1, 1), (0, 0)))          # [66, 66, 64]
        rows = xp[hs0 : hs0 + NROWS]                          # [34, 66, 64]
        win = sliding_window_view(rows, KS, axis=1)           # [34, 64(ws), 64(c), 3(dk1)]
        win = win.transpose(0, 1, 3, 2).reshape(NROWS, W, RW)  # [34, 64, 192]
        dup = np.repeat(win, 2, axis=1)                       # [34, 128, 192]
        out.append(
            np.ascontiguousarray(dup.transpose(1, 0, 2)).reshape(WO, NROWS * RW)
        )
    return out


def _ensure_axon_hooks_module():
    """This image's antenv lacks axon_hooks; run_bass_kernel_spmd imports it
    when BASS_TRACE is set. Provide it (registering the real NTFF hook when
    available) so tracing degrades gracefully instead of crashing."""
    try:
        import antenv.axon_hooks  # noqa: F401
        return
    except ImportError:
        pass
    import sys
    import types

    try:
        import antenv
    except ImportError:
        return
    mod = types.ModuleType("antenv.axon_hooks")
    _hook = [None]
    mod.set_axon_ntff_profile_hook = lambda h: _hook.__setitem__(0, h)
    mod.get_axon_ntff_profile_hook = lambda: _hook[0]
    sys.modules["antenv.axon_hooks"] = mod
    antenv.axon_hooks = mod
    try:
        from trn_agent_boot.trn_boot import _ntff_profile_via_ctypes

        h = _ntff_profile_via_ctypes("/opt/axon/libaxon_pjrt.so")
        if h is not None:
            _hook[0] = h
    except Exception:
        pass


_ensure_axon_hooks_module()

last_results = None  # BassKernelResults of the most recent kernel() call


def kernel(x, meta_w):
    global _CACHED, last_results
    x = np.ascontiguousarray(np.asarray(x, dtype=np.float32))
    meta_w = np.asarray(meta_w, dtype=np.float32)

    if _CACHED is None:
        _CACHED = _build_nc()
    nc = _CACHED

    xrbs = _prep_xrb(x)
    in_maps = []
    for ci in range(N_CORES):
        b, ho0 = ci // CORES_PER_B, (ci % CORES_PER_B) * HO_PC
        mw_c = meta_w[b, ho0 : ho0 + HO_PC]
        if FMAJOR:
            mw_c = np.ascontiguousarray(
                mw_c.reshape(HO_PC, WO, K, F).transpose(0, 1, 3, 2)
            ).reshape(HO_PC, WO, QF)
        in_maps.append({"mw": mw_c, "xrb": xrbs[ci]})

    res = run_bass_kernel_spmd(nc, in_maps, list(range(N_CORES)))
    last_results = res

    out = np.empty((B, HO, WO, F), np.float32)
    for ci in range(N_CORES):
        b, ho0 = ci // CORES_PER_B, (ci % CORES_PER_B) * HO_PC
        o = res.results[ci]["out"].reshape(WO, HO_PC, F)
        out[b, ho0 : ho0 + HO_PC] = o.transpose(1, 0, 2)
    return out



# revision 2
# speedup vs baseline: 1.2858x; 1.2858x over previous
"""Trainium2 Bass kernel for nn_MetaUpSample (2x meta-upsample, 3x3 dynamic filters).

out[b,ho,wo,f] = sum_k patches[b,ho,wo,k] * meta_w[b,ho,wo,k*3+f]
  patches[b,ho,wo,(dk0,dk1,c)] = x_pad[b, ho//2+dk0, wo//2+dk1, c]

Sharding: 8 cores, core ci handles b = ci//2, ho in [(ci%2)*64, (ci%2)*64+64).

v2 design notes (from the 193.7us fp32 baseline's trace):
 - meta_w dominates HBM traffic; the rel-err budget (2e-2) is ~60x wider than
   fp16 rounding on a K=576 dot (~3.5e-4), so the host casts meta_w and the
   patch rows to fp16, halving the stream to ~28.3 MiB/core.
 - the fp32 baseline's [h,w,q]->[w,h,q] DMA made 512 descriptors/tile (6912B
   each); the 16 DMA engines were only ~78% busy (descriptor-feed starved).
   Host now pre-transposes meta_w to w-major [WO, HO_PC, F, K] so each tile is
   128 descriptors of rows*3456B, per-partition contiguous on both sides.
 - scalar_tensor_tensor has no DVE 2x/4x perf mode, so fp16 does not speed the
   multiply-accumulate up; instead the 192 fused ops are split between DVE and
   GpSimd (both implement scalar_tensor_tensor with accum_out), pattern K_PAT.
 - xrb (duplicated patch rows) goes on the ACT ring: head rows first so DVE can
   start after tile 0, bulk deferred until tile 0 lands to not steal DMA
   engines from the critical mw stream at startup.
"""
from contextlib import ExitStack

import numpy as np

import concourse.bass as bass
import concourse.mybir as mybir
from concourse.bass_utils import run_bass_kernel_spmd

B, H, W, C = 4, 64, 64, 64
HO, WO, F = 128, 128, 3
KS = 3
K = KS * KS * C            # 576
QF = K * F                 # 1728 meta_w channels
RW = KS * C                # 192 floats per patch row (dk1, c)
N_CORES = 8
CORES_PER_B = N_CORES // B         # 2
HO_PC = HO // CORES_PER_B          # 64 output rows per core
NHS = HO_PC // 2                   # 32 hs tiles per core
NROWS = NHS + 2                    # 34 cached padded x rows per core

import os

NBUF = int(os.environ.get("K_NBUF", "3"))    # meta_w buffer slots
RPT = int(os.environ.get("K_RPT", "4"))      # meta_w rows per steady DMA tile
NSCR = int(os.environ.get("K_NSCR", "2"))    # per-engine scratch ring slots
# 7-row engine assignment pattern, V=vector(DVE) P=gpsimd(Pool)
PAT = os.environ.get("K_PAT", "VPVPVPV")
XH = 6                     # xrb head rows in the first chunk (covers ramp)

f16 = mybir.dt.float16
f32 = mybir.dt.float32

# ramp head keeps DVE start latency low; small tail drains fast
_head = [1, 1, 2]
_tail = [2, 1, 1]
_mid = (HO_PC - sum(_head) - sum(_tail)) // RPT
SCHED = _head + [RPT] * _mid + _tail
assert sum(SCHED) == HO_PC
NT = len(SCHED)
ROW0 = [sum(SCHED[:t]) for t in range(NT)]          # first ho row of tile t
MAXR = max(SCHED)

# per-row engine assignment
ENG = [PAT[r % len(PAT)] for r in range(HO_PC)]
# cumulative op counts per engine before tile t (3 ops per row)
CUMV = [3 * sum(1 for r in range(ROW0[t]) if ENG[r] == "V") for t in range(NT + 1 - 1)] + [
    3 * sum(1 for r in range(HO_PC) if ENG[r] == "V")
]
CUMP = [3 * sum(1 for r in range(ROW0[t]) if ENG[r] == "P") for t in range(NT + 1 - 1)] + [
    3 * sum(1 for r in range(HO_PC) if ENG[r] == "P")
]
OUT_SPLIT = 56  # rows stored in the early (overlapped) output DMA piece
V_OPS_SPLIT = 3 * sum(1 for r in range(OUT_SPLIT) if ENG[r] == "V")
P_OPS_SPLIT = 3 * sum(1 for r in range(OUT_SPLIT) if ENG[r] == "P")

_CACHED = None


def _build_nc():
    # Cross-engine ordering is explicit via semaphores; same-engine scratch
    # WAW is safe because each engine drains its pipe between ops.
    nc = bass.Bass(detect_race_conditions=False)
    mw_d = nc.declare_dram_parameter("mw", [WO, HO_PC * QF], f16, isOutput=False)
    xrb_d = nc.declare_dram_parameter("xrb", [WO, NROWS * RW], f16, isOutput=False)
    out_d = nc.declare_dram_parameter("out", [WO, HO_PC * F], f32, isOutput=True)

    with ExitStack() as ctx:
        xrow = ctx.enter_context(nc.sbuf_tensor([WO, NROWS * RW], f16))
        mwbuf = ctx.enter_context(nc.sbuf_tensor([WO, NBUF * MAXR * QF], f16))
        scr_v = ctx.enter_context(nc.sbuf_tensor([WO, NSCR * K], f16))
        scr_p = ctx.enter_context(nc.sbuf_tensor([WO, NSCR * K], f16))
        out_sb = ctx.enter_context(nc.sbuf_tensor([WO, HO_PC * F], f32))
        slot_sem = [ctx.enter_context(nc.semaphore(f"slot{j}")) for j in range(NBUF)]
        misc_sem = ctx.enter_context(nc.semaphore("misc"))
        cmp_v = ctx.enter_context(nc.semaphore("cmp_v"))
        cmp_p = ctx.enter_context(nc.semaphore("cmp_p"))
        block = ctx.enter_context(nc.Block())

        def slot_ap(j, rows):
            base = j * MAXR * QF
            return mwbuf[:, base : base + rows * QF]

        @block.sync
        def _(sync):
            for i in range(NT):
                j = i % NBUF
                rows, row0 = SCHED[i], ROW0[i]
                if i >= NBUF:
                    # both engines finished reading the slot's previous tile
                    nxt = i - NBUF + 1
                    if CUMV[nxt]:
                        sync.wait_ge(cmp_v, CUMV[nxt])
                    if CUMP[nxt]:
                        sync.wait_ge(cmp_p, CUMP[nxt])
                sync.dma_start(
                    out=slot_ap(j, rows),
                    in_=mw_d[:, row0 * QF : (row0 + rows) * QF],
                ).then_inc(slot_sem[j], 16)

        @block.scalar
        def _(scalar):
            # head chunk so compute can start right after mw tile 0
            scalar.dma_start(
                out=xrow[:, : XH * RW], in_=xrb_d[:, : XH * RW]
            ).then_inc(misc_sem, 16)
            # bulk deferred past tile 0 so it doesn't steal DMA engines from
            # the mw stream's critical head
            scalar.wait_ge(slot_sem[0], 16)
            scalar.dma_start(
                out=xrow[:, XH * RW :], in_=xrb_d[:, XH * RW :]
            ).then_inc(misc_sem, 16)
            # overlap the bulk of the output store with the tail tiles
            if V_OPS_SPLIT:
                scalar.wait_ge(cmp_v, V_OPS_SPLIT)
            if P_OPS_SPLIT:
                scalar.wait_ge(cmp_p, P_OPS_SPLIT)
            scalar.dma_start(
                out=out_d[:, : OUT_SPLIT * F], in_=out_sb[:, : OUT_SPLIT * F]
            ).then_inc(misc_sem, 16)
            if CUMV[NT]:
                scalar.wait_ge(cmp_v, CUMV[NT])
            if CUMP[NT]:
                scalar.wait_ge(cmp_p, CUMP[NT])
            scalar.dma_start(
                out=out_d[:, OUT_SPLIT * F :], in_=out_sb[:, OUT_SPLIT * F :]
            ).then_inc(misc_sem, 16)

        def compute_body(eng, tag, scr, cmp_sem):
            eng.wait_ge(misc_sem, 16)
            nv = 0
            xrow_full_waited = False
            for i in range(NT):
                j, p = i % NBUF, i // NBUF
                rows, row0 = SCHED[i], ROW0[i]
                my_rows = [r for r in range(row0, row0 + rows) if ENG[r] == tag]
                if not my_rows:
                    continue
                if not xrow_full_waited and (my_rows[-1] // 2) + 2 >= XH:
                    eng.wait_ge(misc_sem, 32)  # rest of xrow loaded
                    xrow_full_waited = True
                eng.wait_ge(slot_sem[j], 16 * (p + 1))
                for ho in my_rows:
                    r = ho - row0
                    win = xrow[:, (ho // 2) * RW : (ho // 2) * RW + KS * RW]
                    for f in range(F):
                        in0 = mwbuf[
                            :,
                            j * MAXR * QF + r * QF + f * K :
                            j * MAXR * QF + r * QF + (f + 1) * K,
                        ]
                        eng.scalar_tensor_tensor(
                            out=scr[:, (nv % NSCR) * K : (nv % NSCR + 1) * K],
                            in0=in0,
                            scalar=1.0,
                            in1=win,
                            op0=mybir.AluOpType.mult,
                            op1=mybir.AluOpType.mult,
                            accum_out=out_sb[:, ho * F + f : ho * F + f + 1],
                        ).then_inc(cmp_sem, 1)
                        nv += 1

        @block.vector
        def _(vector):
            compute_body(vector, "V", scr_v, cmp_v)

        @block.gpsimd
        def _(gpsimd):
            compute_body(gpsimd, "P", scr_p, cmp_p)

    return nc


def _prep_xrb(x):
    """Per-core duplicated patch-row tensors (fp16).

    xrb[ci][wo, hpl*RW + dk1*C + c] = x_pad[b, hs0+hpl, wo//2 + dk1, c]
    where x_pad has 1 zero row/col of padding on each side.
    """
    from numpy.lib.stride_tricks import sliding_window_view

    out = []
    for ci in range(N_CORES):
        b, hs0 = ci // CORES_PER_B, (ci % CORES_PER_B) * NHS
        xp = np.pad(x[b], ((1, 1), (1, 1), (0, 0)))          # [66, 66, 64]
        rows = xp[hs0 : hs0 + NROWS]                          # [34, 66, 64]
        win = sliding_window_view(rows, KS, axis=1)           # [34, 64(ws), 64(c), 3(dk1)]
        win = win.transpose(0, 1, 3, 2).reshape(NROWS, W, RW)  # [34, 64, 192]
        dup = np.repeat(win, 2, axis=1)                       # [34, 128, 192]
        out.append(
            np.ascontiguousarray(dup.transpose(1, 0, 2))
            .reshape(WO, NROWS * RW)
            .astype(np.float16)
        )
    return out


def _ensure_axon_hooks_module():
    """This image's antenv lacks axon_hooks; run_bass_kernel_spmd imports it
    when BASS_TRACE is set. Provide it (registering the real NTFF hook when
    available) so tracing degrades gracefully instead of crashing."""
    try:
        import antenv.axon_hooks  # noqa: F401
        return
    except ImportError:
        pass
    import sys
    import types

    try:
        import antenv
    except ImportError:
        return
    mod = types.ModuleType("antenv.axon_hooks")
    _hook = [None]
    mod.set_axon_ntff_profile_hook = lambda h: _hook.__setitem__(0, h)
    mod.get_axon_ntff_profile_hook = lambda: _hook[0]
    sys.modules["antenv.axon_hooks"] = mod
    antenv.axon_hooks = mod
    try:
        from trn_agent_boot.trn_boot import _ntff_profile_via_ctypes

        h = _ntff_profile_via_ctypes("/opt/axon/libaxon_pjrt.so")
        if h is not None:
            _hook[0] = h
    except Exception:
        pass


_ensure_axon_hooks_module()

last_results = None  # BassKernelResults of the most recent kernel() call


def kernel(x, meta_w):
    global _CACHED, last_results
    x = np.ascontiguousarray(np.asarray(x, dtype=np.float32))
    meta_w = np.asarray(meta_w, dtype=np.float32)

    if _CACHED is None:
        _CACHED = _build_nc()
    nc = _CACHED

    xrbs = _prep_xrb(x)
    in_maps = []
    for ci in range(N_CORES):
        b, ho0 = ci // CORES_PER_B, (ci % CORES_PER_B) * HO_PC
        # w-major + f-major fp16: [WO, HO_PC, F, K] flattened
        mw_c = (
            meta_w[b, ho0 : ho0 + HO_PC]
            .reshape(HO_PC, WO, K, F)
            .transpose(1, 0, 3, 2)
            .astype(np.float16)
            .reshape(WO, HO_PC * QF)
        )
        in_maps.append({"mw": mw_c, "xrb": xrbs[ci]})

    res = run_bass_kernel_spmd(nc, in_maps, list(range(N_CORES)))
    last_results = res

    out = np.empty((B, HO, WO, F), np.float32)
    for ci in range(N_CORES):
        b, ho0 = ci // CORES_PER_B, (ci % CORES_PER_B) * HO_PC
        o = res.results[ci]["out"].reshape(WO, HO_PC, F)
        out[b, ho0 : ho0 + HO_PC] = o.transpose(1, 0, 2)
    return out
